# revision 1
# baseline (speedup 1.0000x reference)
"""Trainium2 Bass kernel for the GNN message-passing problem.

Math notes (why this is exact, not an approximation):
  score[i,b,j] = q[i,b]@wQKq + k[j,b]@wQKk + bQK.  Softmax over j is
  invariant to terms constant in j, so the attention weights are
  p[b,j] = softmax_j(nodes[j,b]@(WK.T@wQKk)) -- independent of the query
  node i.  Hence every node receives the same aggregated message and all
  Nn nodes are identical after one communicate() round; the final
  max-over-nodes is the common value.  Since p sums to 1 and V/A2N are
  affine, aggre@WV.T@WA.T collapses to hbar@(WA@WV).T with
  hbar[b] = sum_j p[b,j]*nodes[j,b].  Rounds 2-4 operate on identical
  nodes (uniform softmax == identity mix) and reduce to [B,H] math.

Execution strategy: a profiled 8-core batch-sharded version spent
~220us of its 288us in the cross-core AllGather needed by the
training-mode BatchNorm (collective mechanism floor + launch skew).
The whole problem fits one core comfortably (~110us of fp32 PE work),
so this version runs the ENTIRE batch on a single NeuronCore with no
collectives at all: per-core time is then pure compute, independent of
inter-core launch skew.

On-chip layout: transposed activations [feature, col] with two
64-feature blocks on the 128 partitions.  Block A (partitions 0-63)
holds batches 0-15, block B (partitions 64-127) batches 16-31;
1536 columns per batch (1535 input nodes + 1 unused scratch column --
the out_enc node is handled out-of-band so the big reductions run on
a clean 1535-column span).

The [18, 24576] fc1 input (x||y, both blocks) is assembled on the HOST
so the device DMA is fully contiguous (the naive per-core transposed
load was 4-byte-strided and cost ~13us per batch-slice).
"""

import numpy as np

try:
    import concourse.bass as bass  # noqa: F401
except ImportError:  # pragma: no cover - container default path
    import sys

    sys.path.insert(0, "/opt/trn_rl_repo")
    import concourse.bass as bass  # noqa: F401

import concourse.bacc as bacc
import concourse.tile as tile
from concourse import mybir
from concourse import bass_utils

F32 = mybir.dt.float32
BF16 = mybir.dt.bfloat16
AF = mybir.ActivationFunctionType
ALU = mybir.AluOpType

B = 32
BL = B // 2  # 16 batches per 64-partition block
N = 1535  # input nodes
NN = N + 1  # +1 col per batch (scratch; out_enc handled separately)
H = 64
R = BL * NN  # 24576 free columns
CH = 512  # PSUM-bank-sized matmul chunk
BN_EPS = 1e-5

# consts tensor column map ([128, CW] fp32)
C_W2 = 0  # [128,128] blockdiag(W2.T, W2.T)
C_WKE = 128  # [128,128] blockdiag(wk x ones, wk x ones)
C_W1 = 256  # rows 0-8 cols 0-63 = W1.T ; rows 9-17 cols 64-127 = W1.T
C_WVA = 384  # [128,64] rows 0-63 = (WA@WV).T ; rows 64-127 = same
C_WENC = 448  # [128,64] rows 0-7 = Wenc.T
C_HEADS = 512  # [64,2] col0 = Wmu[0], col1 = Wsig[0]
C_B1 = 514  # [128,1] concat(b1,b1)
C_B2 = 515  # [128,1] concat(b2,b2)
C_GAMMA = 516  # [64,1]
C_BETA = 517  # [64,1]
C_BVA = 518  # [64,1] WA@bV + bA
C_BENC = 519  # [128,1] concat(benc,benc)
C_BMU = 520  # row0 = bmu
C_BSIG = 521  # row0 = bsig
C_WKEB = 522  # [128,64] fp32-packed bf16 blockdiag WKE ([128,128] bf16)
CW = 586

# score matmul mode: "bf16_view" reads the fp32 h2 buffer through a stride-2
# bf16 AP (high half-words == truncated fp32) at 1 PE cycle/col; "fp32" is the
# 4 cycle/col fallback.  The softmax path tolerates bf16 scores (adds ~3e-5
# output error, measured).
SCORE_MODE = "bf16_view"


def build_nc(stage=99, score_mode=None):
    """stage caps how much of the pipeline is emitted (debug bisect aid);
    the output tensor is written with whatever is available at that stage."""
    if score_mode is None:
        score_mode = SCORE_MODE
    nc = bacc.Bacc(
        "TRN2",
        target_bir_lowering=False,
        debug=False,
        enable_asserts=True,
        num_devices=1,
    )
    xcat = nc.dram_tensor("xcat", [18, R], F32, kind="ExternalInput").ap()
    oxxT = nc.dram_tensor("oxxT", [8, B], F32, kind="ExternalInput").ap()
    consts = nc.dram_tensor("consts", [128, CW], F32, kind="ExternalInput").ap()
    out = nc.dram_tensor("out", [1, 2 * B], F32, kind="ExternalOutput").ap()

    with tile.TileContext(nc) as tc:
        with (
            tc.tile_pool(name="big", bufs=1) as big,
            tc.tile_pool(name="small", bufs=1) as small,
            tc.tile_pool(name="h1p", bufs=3) as h1p,
            tc.tile_pool(name="h2p", bufs=3) as h2p,
            tc.tile_pool(name="ep", bufs=3) as ep,
            tc.tile_pool(name="prodp", bufs=3) as prodp,
            tc.tile_pool(name="psum_big", bufs=6, space="PSUM") as psum_big,
            tc.tile_pool(name="psum_small", bufs=2, space="PSUM") as psum_small,
        ):
          def _body():
            # ---- input DMAs ----
            consts_sb = big.tile([128, CW], F32, tag="consts")
            # ordered by when the main loop needs them: W1 (fc1), the
            # bias/WKEB strip (relu1 + score), W2, then the post-loop blocks
            xT = big.tile([18, R], F32, tag="xT")
            nc.sync.dma_start(
                out=consts_sb[:, C_W1 : C_W1 + 128], in_=consts[:, C_W1 : C_W1 + 128]
            )
            nc.sync.dma_start(out=xT[:, 0:NN], in_=xcat[:, 0:NN])
            nc.sync.dma_start(
                out=consts_sb[:, C_HEADS:CW], in_=consts[:, C_HEADS:CW]
            )
            nc.sync.dma_start(
                out=consts_sb[:, C_W2 : C_W2 + 128], in_=consts[:, C_W2 : C_W2 + 128]
            )
            for s in range(1, BL):
                sp = slice(s * NN, (s + 1) * NN)
                nc.sync.dma_start(out=xT[:, sp], in_=xcat[:, sp])
            nc.sync.dma_start(
                out=consts_sb[:, C_WKE : C_WKE + 128],
                in_=consts[:, C_WKE : C_WKE + 128],
            )
            nc.sync.dma_start(
                out=consts_sb[:, C_WVA : C_HEADS], in_=consts[:, C_WVA : C_HEADS]
            )

            oxx_sb = small.tile([8, B], F32, tag="oxx")
            nc.sync.dma_start(out=oxx_sb[:], in_=oxxT)

            # ---- main pipeline: one superchunk per batch-pair (1536 cols) ----
            dall = small.tile([128, BL], F32, tag="dall")
            numall = small.tile([128, BL], F32, tag="numall")
            w1blk = consts_sb[0:18, C_W1 : C_W1 + 128]
            w2blk = consts_sb[:, C_W2 : C_W2 + 128]
            if score_mode == "bf16_view":
                wkeblk = consts_sb[:, C_WKEB : C_WKEB + 64].bitcast(BF16)
            else:
                wkeblk = consts_sb[:, C_WKE : C_WKE + 128]
            for s in range(BL):
                base = s * NN
                # [128,512] single-bank PSUM tiles throughout: walrus rejects
                # multi-bank matmul outputs, and engine reads that span PSUM
                # banks fault the device.
                h1 = h1p.tile([128, NN], F32, tag="h1")
                for c in range(3):
                    sp = slice(c * CH, (c + 1) * CH)
                    ps = psum_big.tile([128, CH], F32, tag="mm")
                    nc.tensor.matmul(
                        ps[:],
                        w1blk,
                        xT[:, base + c * CH : base + (c + 1) * CH],
                        start=True,
                        stop=True,
                    )
                    nc.scalar.activation(
                        out=h1[:, sp],
                        in_=ps[:],
                        func=AF.Relu,
                        bias=consts_sb[:, C_B1 : C_B1 + 1],
                        scale=1.0,
                    )
                h2 = h2p.tile([128, NN], F32, tag="h2")
                for c in range(3):
                    sp = slice(c * CH, (c + 1) * CH)
                    ps = psum_big.tile([128, CH], F32, tag="mm")
                    nc.tensor.matmul(ps[:], w2blk, h1[:, sp], start=True, stop=True)
                    nc.scalar.activation(
                        out=h2[:, sp],
                        in_=ps[:],
                        func=AF.Relu,
                        bias=consts_sb[:, C_B2 : C_B2 + 1],
                        scale=1.0,
                    )
                if score_mode == "bf16_view":
                    # [128, 1, NN] bf16 view selecting each fp32's high half
                    h2mm = h2[:].bitcast(BF16).rearrange(
                        "p (n two) -> p two n", two=2
                    )[:, 1:2, :]
                else:
                    h2mm = h2[:].unsqueeze(1)
                ebc = ep.tile([128, NN], F32, tag="ebc")
                for c in range(3):
                    sp = slice(c * CH, (c + 1) * CH)
                    ps = psum_big.tile([128, CH], F32, tag="mm")
                    nc.tensor.matmul(
                        ps[:], wkeblk, h2mm[:, :, sp], start=True, stop=True
                    )
                    nc.scalar.activation(
                        out=ebc[:, sp], in_=ps[:], func=AF.Exp, bias=0.0, scale=1.0
                    )
                # per-batch softmax denominator and weighted node sum
                # (cols 0..1534 only; col 1535 is scratch)
                nc.vector.tensor_reduce(
                    out=dall[:, s : s + 1],
                    in_=ebc[:, 0:N],
                    axis=mybir.AxisListType.X,
                    op=ALU.add,
                )
                prod = prodp.tile([128, NN], F32, tag="prod")
                nc.vector.tensor_mul(prod[:, 0:N], h2[:, 0:N], ebc[:, 0:N])
                nc.vector.tensor_reduce(
                    out=numall[:, s : s + 1],
                    in_=prod[:, 0:N],
                    axis=mybir.AxisListType.X,
                    op=ALU.add,
                )

            if stage == 1:
                nc.sync.dma_start(out=out, in_=dall[0:1, 0 : 2 * B])
                return

            # ---- out_enc encoder: henc = relu(oxx @ Wenc.T + benc) [128, BL]
            # (emitted AFTER the main loop: engine queues dispatch in program
            # order, and putting these first stalled fc1 behind the late
            # consts DMAs for ~7us)
            enc_ps = psum_small.tile([128, BL], F32, tag="sps")
            wenc = consts_sb[0:8, C_WENC : C_WENC + 64]
            nc.tensor.matmul(
                enc_ps[0:64, :], wenc, oxx_sb[:, 0:BL], start=True, stop=True
            )
            nc.tensor.matmul(
                enc_ps[64:128, :], wenc, oxx_sb[:, BL:B], start=True, stop=True
            )
            henc = small.tile([128, BL], F32, tag="henc")
            nc.scalar.activation(
                out=henc[:],
                in_=enc_ps[:],
                func=AF.Relu,
                bias=consts_sb[:, C_BENC : C_BENC + 1],
                scale=1.0,
            )
            # e_enc = exp(wk . henc), replicated per block partition
            ee_ps = psum_small.tile([128, BL], F32, tag="sps")
            nc.tensor.matmul(
                ee_ps[:],
                consts_sb[:, C_WKE : C_WKE + 128],
                henc[:],
                start=True,
                stop=True,
            )
            eenc = small.tile([128, BL], F32, tag="eenc")
            nc.scalar.activation(
                out=eenc[:], in_=ee_ps[:], func=AF.Exp, bias=0.0, scale=1.0
            )
            prod_enc = small.tile([128, BL], F32, tag="prod_enc")
            nc.vector.tensor_mul(prod_enc[:], henc[:], eenc[:])

            # add the out_enc node's contribution, then hbar = num / d
            dtot = small.tile([128, BL], F32, tag="dtot")
            numtot = small.tile([128, BL], F32, tag="numtot")
            nc.vector.tensor_add(dtot[:], dall[:], eenc[:])
            nc.vector.tensor_add(numtot[:], numall[:], prod_enc[:])
            rd = small.tile([128, BL], F32, tag="rd")
            nc.vector.reciprocal(out=rd[:], in_=dtot[:])
            hbar = small.tile([128, BL], F32, tag="hbar")
            nc.vector.tensor_mul(hbar[:], numtot[:], rd[:])

            if stage == 2:
                nc.sync.dma_start(out=out[0:1, 0:BL], in_=hbar[0:1, :])
                return

            # ---- z = hbar @ (WA@WV).T + bva   -> [64, B] ----
            va0 = consts_sb[0:64, C_WVA : C_WVA + 64]
            va1 = consts_sb[64:128, C_WVA : C_WVA + 64]
            z_ps = psum_small.tile([64, B], F32, tag="sps")
            nc.tensor.matmul(z_ps[:, 0:BL], va0, hbar[0:64, :], start=True, stop=True)
            nc.tensor.matmul(
                z_ps[:, BL:B], va1, hbar[64:128, :], start=True, stop=True
            )
            bva = consts_sb[0:64, C_BVA : C_BVA + 1]
            zT = small.tile([64, B], F32, tag="zT")
            nc.scalar.activation(
                out=zT[:], in_=z_ps[:], func=AF.Identity, bias=bva, scale=1.0
            )

            if stage == 3:
                nc.sync.dma_start(out=out[0:1, 0:B], in_=zT[0:1, :])
                return

            # ---- tail: 4x (BN + relu), 3x linear, heads ----
            eps_col = small.tile([64, 1], F32, tag="eps")
            nc.vector.memset(eps_col[:], BN_EPS)
            gamma = consts_sb[0:64, C_GAMMA : C_GAMMA + 1]
            beta = consts_sb[0:64, C_BETA : C_BETA + 1]

            cur = zT
            node = None
            for r in range(4):
                st6 = small.tile([64, 6], F32, tag=f"st6_{r}")
                mv = small.tile([64, 2], F32, tag=f"mv_{r}")
                nc.vector.bn_stats(out=st6[:], in_=cur[:])
                nc.vector.bn_aggr(out=mv[:], in_=st6[:])
                sd = small.tile([64, 1], F32, tag=f"sd_{r}")
                nc.scalar.activation(
                    out=sd[:], in_=mv[:, 1:2], func=AF.Sqrt, bias=eps_col[:], scale=1.0
                )
                rstd = small.tile([64, 1], F32, tag=f"rstd_{r}")
                nc.vector.reciprocal(out=rstd[:], in_=sd[:])
                a = small.tile([64, 1], F32, tag=f"a_{r}")
                nc.vector.tensor_mul(a[:], rstd[:], gamma)
                mc = small.tile([64, 1], F32, tag=f"mc_{r}")
                nc.vector.tensor_mul(mc[:], mv[:, 0:1], a[:])
                cb = small.tile([64, 1], F32, tag=f"cb_{r}")
                nc.vector.tensor_sub(cb[:], beta, mc[:])
                node = small.tile([64, B], F32, tag=f"node_{r}")
                nc.scalar.activation(
                    out=node[:], in_=cur[:], func=AF.Relu, bias=cb[:], scale=a[:]
                )
                if r < 3:
                    zp = psum_small.tile([64, B], F32, tag="sps")
                    nc.tensor.matmul(zp[:], va0, node[:], start=True, stop=True)
                    nxt = small.tile([64, B], F32, tag=f"z_{r + 1}")
                    nc.scalar.activation(
                        out=nxt[:], in_=zp[:], func=AF.Identity, bias=bva, scale=1.0
                    )
                    cur = nxt

            # ---- heads (everything on partition 0: mu cols 0-31, sig 32-63) ----
            hp_mu = psum_small.tile([1, B], F32, tag="sps")
            nc.tensor.matmul(
                hp_mu[:],
                consts_sb[0:64, C_HEADS : C_HEADS + 1],
                node[:],
                start=True,
                stop=True,
            )
            hp_sig = psum_small.tile([1, B], F32, tag="sps")
            nc.tensor.matmul(
                hp_sig[:],
                consts_sb[0:64, C_HEADS + 1 : C_HEADS + 2],
                node[:],
                start=True,
                stop=True,
            )
            out_sb = small.tile([1, 2 * B], F32, tag="out_sb")
            nc.scalar.activation(
                out=out_sb[0:1, 0:B],
                in_=hp_mu[:],
                func=AF.Identity,
                bias=consts_sb[0:1, C_BMU : C_BMU + 1],
                scale=1.0,
            )
            sig_t = small.tile([1, B], F32, tag="sig_t")
            nc.scalar.activation(
                out=sig_t[:],
                in_=hp_sig[:],
                func=AF.Square,
                bias=consts_sb[0:1, C_BSIG : C_BSIG + 1],
                scale=1.0,
            )
            nc.vector.tensor_scalar_add(out_sb[0:1, B : 2 * B], sig_t[:], 0.01)
            nc.sync.dma_start(out=out, in_=out_sb[:])

          _body()

    nc.compile()
    return nc


def make_consts(inp):
    f32 = np.float32
    W1 = np.asarray(inp["W1"], f32)
    b1 = np.asarray(inp["b1"], f32)
    W2 = np.asarray(inp["W2"], f32)
    b2 = np.asarray(inp["b2"], f32)
    Wenc = np.asarray(inp["Wenc"], f32)
    benc = np.asarray(inp["benc"], f32)
    WK = np.asarray(inp["WK"], f32)
    WV = np.asarray(inp["WV"], f32)
    bV = np.asarray(inp["bV"], f32)
    wQKk = np.asarray(inp["wQKk"], f32)
    WA = np.asarray(inp["WA"], f32)
    bA = np.asarray(inp["bA"], f32)
    gamma = np.asarray(inp["gamma"], f32)
    beta = np.asarray(inp["beta"], f32)
    Wmu = np.asarray(inp["Wmu"], f32)
    bmu = np.asarray(inp["bmu"], f32)
    Wsig = np.asarray(inp["Wsig"], f32)
    bsig = np.asarray(inp["bsig"], f32)

    wk = WK.T @ wQKk  # [H]
    Wva = WA @ WV  # [H,H]
    bva = WA @ bV + bA

    c = np.zeros((128, CW), f32)
    c[0:64, C_W2 : C_W2 + 64] = W2.T
    c[64:128, C_W2 + 64 : C_W2 + 128] = W2.T
    c[0:64, C_WKE : C_WKE + 64] = wk[:, None]
    c[64:128, C_WKE + 64 : C_WKE + 128] = wk[:, None]
    c[0:9, C_W1 : C_W1 + 64] = W1.T
    c[9:18, C_W1 + 64 : C_W1 + 128] = W1.T
    c[0:64, C_WVA : C_WVA + 64] = Wva.T
    c[64:128, C_WVA : C_WVA + 64] = Wva.T
    c[0:8, C_WENC : C_WENC + 64] = Wenc.T
    c[0:64, C_HEADS] = Wmu[0]
    c[0:64, C_HEADS + 1] = Wsig[0]
    c[0:64, C_B1] = b1
    c[64:128, C_B1] = b1
    c[0:64, C_B2] = b2
    c[64:128, C_B2] = b2
    c[0:64, C_GAMMA] = gamma
    c[0:64, C_BETA] = beta
    c[0:64, C_BVA] = bva
    c[0:64, C_BENC] = benc
    c[64:128, C_BENC] = benc
    c[0, C_BMU] = bmu[0]
    c[0, C_BSIG] = bsig[0]

    # bf16 blockdiag WKE, packed 2 bf16 per fp32 column (little-endian)
    wkeb = np.zeros((128, 128), np.float32)
    wkeb[0:64, 0:64] = wk[:, None]
    wkeb[64:128, 64:128] = wk[:, None]
    u = wkeb.view(np.uint32)
    b16 = ((u + 0x7FFF + ((u >> 16) & 1)) >> 16).astype(np.uint32)  # rne
    packed = (b16[:, 0::2] | (b16[:, 1::2] << 16)).astype(np.uint32)
    c[:, C_WKEB : C_WKEB + 64] = packed.view(np.float32)
    return c


def make_in_maps(inputs):
    f32 = np.float32
    xx = np.asarray(inputs["input_xx"], f32)  # [B, N, 8]
    yy = np.asarray(inputs["input_yy"], f32)  # [B, N]
    oxx = np.asarray(inputs["output_xx"], f32)  # [B, 1, 8]

    xcat = np.zeros((18, BL, NN), f32)
    xcat[0:8, :, 0:N] = xx[0:BL].transpose(2, 0, 1)
    xcat[8, :, 0:N] = yy[0:BL]
    xcat[9:17, :, 0:N] = xx[BL:B].transpose(2, 0, 1)
    xcat[17, :, 0:N] = yy[BL:B]
    xcat = np.ascontiguousarray(xcat.reshape(18, R))

    oxxT = np.ascontiguousarray(oxx[:, 0, :].T)  # [8, B]
    consts = make_consts(inputs)
    return [{"xcat": xcat, "oxxT": oxxT, "consts": consts}]


_NC_CACHE = {}


def get_nc():
    if "nc" not in _NC_CACHE:
        _NC_CACHE["nc"] = build_nc()
    return _NC_CACHE["nc"]


def kernel(**inputs):
    nc = get_nc()
    in_maps = make_in_maps(inputs)
    res = bass_utils.run_bass_kernel_spmd(nc, in_maps, core_ids=[0])
    out = np.asarray(res.results[0]["out"], np.float32).reshape(2 * B)
    mu_out = out[0:B].reshape(B, 1).copy()
    sig_out = out[B : 2 * B].reshape(B, 1).copy()
    return mu_out, sig_out



# revision 16
# speedup vs baseline: 1.3956x; 1.3956x over previous
"""Trainium2 Bass kernel for the GNN message-passing problem.

Math notes (why this is exact, not an approximation):
  score[i,b,j] = q[i,b]@wQKq + k[j,b]@wQKk + bQK.  Softmax over j is
  invariant to terms constant in j, so the attention weights are
  p[b,j] = softmax_j(nodes[j,b]@(WK.T@wQKk)) -- independent of the query
  node i.  Hence every node receives the same aggregated message and all
  Nn nodes are identical after one communicate() round; the final
  max-over-nodes is the common value.  Since p sums to 1 and V/A2N are
  affine, aggre@WV.T@WA.T collapses to hbar@(WA@WV).T with
  hbar[b] = sum_j p[b,j]*nodes[j,b].  Rounds 2-4 operate on identical
  nodes (uniform softmax == identity mix) and reduce to [B,H] math.

Execution strategy: single NeuronCore.  Multi-core was measured and
rejected: an 8-core (or even 2-core) AllReduce/AllGather through the
collectives runtime has a ~70-90us mechanism floor on this setup --
more than this whole kernel.

Perf structure (vs the 160us fp32 predecessor):
  * All three big matmul passes (fc1, fc2, score) run in float32r --
    1 PE cycle/column instead of 4 for fp32 (ap>=256), at ~1.6e-4
    matmul error (measured) vs ~4e-3 for bf16.  h1/h2 live in f32r
    tiles; their producers round on write, which walrus requires.
  * Softmax denominator rides the Exp drain for free via activation
    accum_out; the numerator sum_j ebc*h2 is one fused DVE
    tensor_tensor_reduce pass instead of tensor_mul + tensor_reduce.
  * PSUM drains are spread across the three non-PE compute engines so
    no engine exceeds ~2.6us per 1536-col superchunk: Scalar does the
    3 Exp chunks, DVE does one fc2-relu chunk + the fused reduce,
    Pool/gpsimd does the other 5 relu chunks (scalar_tensor_tensor:
    relu(psum + bias) = (ps add b) max zeros).

On-chip layout: transposed activations [feature, col] with two
64-feature blocks on the 128 partitions.  Block A (partitions 0-63)
holds batches 0-15, block B (partitions 64-127) batches 16-31;
1536 columns per batch (1535 input nodes + 1 unused scratch column --
the out_enc node is handled out-of-band so the big reductions run on
a clean 1535-column span).

The [18, 24576] fc1 input (x||y, both blocks) is assembled on the HOST
so the device DMA is fully contiguous.
"""

import numpy as np

try:
    import concourse.bass as bass  # noqa: F401
except ImportError:  # pragma: no cover - container default path
    import sys

    sys.path.insert(0, "/opt/trn_rl_repo")
    import concourse.bass as bass  # noqa: F401

import concourse.bacc as bacc
import concourse.tile as tile
from concourse import mybir
from concourse import bass_utils

F32 = mybir.dt.float32
F32R = mybir.dt.float32r
AF = mybir.ActivationFunctionType
ALU = mybir.AluOpType

B = 32
BL = B // 2  # 16 batches per 64-partition block
N = 1535  # input nodes
NN = N + 1  # +1 col per batch (scratch; out_enc handled separately)
H = 64
R = BL * NN  # 24576 free columns
CH = 512  # PSUM-bank-sized matmul chunk
BN_EPS = 1e-5

# f32r consts tensor column map ([128, CWR])
CR_W1 = 0  # rows 0-8 cols 0-63 = W1.T ; rows 9-17 cols 64-127 = W1.T
CR_W2 = 128  # [128,128] blockdiag(W2.T, W2.T)
CR_WKE = 256  # [128,128] blockdiag(wk x ones, wk x ones)
CWR = 384

# fp32 consts tensor column map ([128, CW])
C_WVA = 0  # [128,64] rows 0-63 = (WA@WV).T ; rows 64-127 = same
C_WENC = 64  # [128,64] rows 0-7 = Wenc.T
C_WKEF = 128  # [128,128] fp32 blockdiag(wk x ones) for the enc path
C_HEADS = 256  # [64,2] col0 = Wmu[0], col1 = Wsig[0]
C_B1 = 258  # [128,1] concat(b1,b1)
C_B2 = 259  # [128,1] concat(b2,b2)
C_GAMMA = 260  # [64,1]
C_BETA = 261  # [64,1]
C_BVA = 262  # [64,1] WA@bV + bA
C_BENC = 263  # [128,1] concat(benc,benc)
C_BMU = 264  # row0 = bmu
C_BSIG = 265  # row0 = bsig
CW = 266


def build_nc(stage=99, use_ttr=True, use_acc=True, use_stt=True, nsc=BL):
    """stage caps how much of the pipeline is emitted (debug bisect aid);
    the output tensor is written with whatever is available at that stage.
    use_ttr/use_acc disable the fused DVE reduce / activation accum_out."""
    nc = bacc.Bacc(
        "TRN2",
        target_bir_lowering=False,
        debug=False,
        enable_asserts=True,
        num_devices=1,
    )
    xcat = nc.dram_tensor("xcat", [18, R], F32R, kind="ExternalInput").ap()
    oxxT = nc.dram_tensor("oxxT", [8, B], F32, kind="ExternalInput").ap()
    constsr = nc.dram_tensor("constsr", [128, CWR], F32R, kind="ExternalInput").ap()
    consts = nc.dram_tensor("consts", [128, CW], F32, kind="ExternalInput").ap()
    out = nc.dram_tensor("out", [1, 2 * B], F32, kind="ExternalOutput").ap()

    with tile.TileContext(nc) as tc:
        with (
            tc.tile_pool(name="big", bufs=1) as big,
            tc.tile_pool(name="small", bufs=1) as small,
            tc.tile_pool(name="h1p", bufs=3) as h1p,
            tc.tile_pool(name="h2p", bufs=3) as h2p,
            tc.tile_pool(name="ep", bufs=3) as ep,
            tc.tile_pool(name="prodp", bufs=2) as prodp,
            tc.tile_pool(name="psum_big", bufs=6, space="PSUM") as psum_big,
            tc.tile_pool(name="psum_small", bufs=2, space="PSUM") as psum_small,
        ):
          def _body():
            # ---- input DMAs (ordered by when the main loop needs them) ----
            constsr_sb = big.tile([128, CWR], F32R, tag="constsr")
            consts_sb = big.tile([128, CW], F32, tag="consts")
            xT = big.tile([18, R], F32R, tag="xT")
            nc.sync.dma_start(
                out=constsr_sb[:, CR_W1 : CR_W1 + 128],
                in_=constsr[:, CR_W1 : CR_W1 + 128],
            )
            nc.sync.dma_start(
                out=consts_sb[:, C_HEADS:CW], in_=consts[:, C_HEADS:CW]
            )
            nc.sync.dma_start(out=xT[:, 0:NN], in_=xcat[:, 0:NN])
            nc.sync.dma_start(
                out=constsr_sb[:, CR_W2 : CR_W2 + 128],
                in_=constsr[:, CR_W2 : CR_W2 + 128],
            )
            nc.sync.dma_start(
                out=constsr_sb[:, CR_WKE : CR_WKE + 128],
                in_=constsr[:, CR_WKE : CR_WKE + 128],
            )
            for s in range(1, BL):
                sp = slice(s * NN, (s + 1) * NN)
                nc.sync.dma_start(out=xT[:, sp], in_=xcat[:, sp])
            nc.sync.dma_start(
                out=consts_sb[:, C_WVA:C_HEADS], in_=consts[:, C_WVA:C_HEADS]
            )
            oxx_sb = small.tile([8, B], F32, tag="oxx")
            nc.sync.dma_start(out=oxx_sb[:], in_=oxxT)

            # ---- main pipeline: one superchunk per batch-pair (1536 cols) ----
            zeros = small.tile([128, CH], F32, tag="zeros")
            nc.vector.memset(zeros[:], 0.0)
            dacc = small.tile([128, 3 * BL], F32, tag="dacc")
            numall = small.tile([128, BL], F32, tag="numall")
            w1blk = constsr_sb[0:18, CR_W1 : CR_W1 + 128]
            w2blk = constsr_sb[:, CR_W2 : CR_W2 + 128]
            wkeblk = constsr_sb[:, CR_WKE : CR_WKE + 128]
            b1 = consts_sb[:, C_B1 : C_B1 + 1]
            b2 = consts_sb[:, C_B2 : C_B2 + 1]
            for s in range(nsc):
                base = s * NN
                # [128,512] single-bank PSUM tiles throughout: walrus rejects
                # multi-bank matmul outputs, and engine reads that span PSUM
                # banks fault the device.
                h1 = h1p.tile([128, NN], F32R, tag="h1")
                for c in range(3):
                    sp = slice(c * CH, (c + 1) * CH)
                    ps = psum_big.tile([128, CH], F32, tag="mm")
                    nc.tensor.matmul(
                        ps[:],
                        w1blk,
                        xT[:, base + c * CH : base + (c + 1) * CH],
                        start=True,
                        stop=True,
                    )
                    # drains split between DVE and Scalar (only engines with
                    # PSUM access): Scalar also carries the 3 Exp chunks, so
                    # it takes 2 relu chunks and DVE takes 4 + the fused num
                    if c < 2 and use_stt:
                        nc.vector.scalar_tensor_tensor(
                            out=h1[:, sp], in0=ps[:], scalar=b1, in1=zeros[:],
                            op0=ALU.add, op1=ALU.max,
                        )
                    else:
                        nc.scalar.activation(
                            out=h1[:, sp], in_=ps[:], func=AF.Relu,
                            bias=b1, scale=1.0,
                        )
                h2 = h2p.tile([128, NN], F32R, tag="h2")
                for c in range(3):
                    sp = slice(c * CH, (c + 1) * CH)
                    ps = psum_big.tile([128, CH], F32, tag="mm")
                    nc.tensor.matmul(ps[:], w2blk, h1[:, sp], start=True, stop=True)
                    # gpsimd/Pool cannot read PSUM; split drains DVE/Scalar
                    if c < 2 and use_stt:
                        nc.vector.scalar_tensor_tensor(
                            out=h2[:, sp], in0=ps[:], scalar=b2, in1=zeros[:],
                            op0=ALU.add, op1=ALU.max,
                        )
                    else:
                        nc.scalar.activation(
                            out=h2[:, sp], in_=ps[:], func=AF.Relu,
                            bias=b2, scale=1.0,
                        )
                ebc = ep.tile([128, NN], F32, tag="ebc")
                for c in range(3):
                    ncol = CH if c < 2 else CH - 1  # skip the scratch col 1535
                    sp = slice(c * CH, c * CH + ncol)
                    ps = psum_big.tile([128, CH], F32, tag="mm")
                    nc.tensor.matmul(
                        ps[:], wkeblk, h2[:, c * CH : (c + 1) * CH],
                        start=True, stop=True,
                    )
                    if use_acc:
                        nc.scalar.activation(
                            out=ebc[:, sp], in_=ps[:, 0:ncol], func=AF.Exp,
                            bias=0.0, scale=1.0,
                            accum_out=dacc[:, 3 * s + c : 3 * s + c + 1],
                        )
                    else:
                        nc.scalar.activation(
                            out=ebc[:, sp], in_=ps[:, 0:ncol], func=AF.Exp,
                            bias=0.0, scale=1.0,
                        )
                if not use_acc:
                    nc.vector.tensor_reduce(
                        out=dacc[:, 3 * s : 3 * s + 1], in_=ebc[:, 0:N],
                        axis=mybir.AxisListType.X, op=ALU.add,
                    )
                    nc.vector.memset(dacc[:, 3 * s + 1 : 3 * s + 3], 0.0)
                # num[b] = sum_j ebc[j]*h2[j] in one fused DVE pass.
                # NOTE: tensor_tensor_reduce faults the HW here (measured);
                # scalar_tensor_tensor with accum_out is the working fused
                # form: out = (h2 * 1.0) * ebc, accum_out = sum(out).
                prod = prodp.tile([128, N], F32, tag="prod")
                if use_ttr:
                    nc.vector.scalar_tensor_tensor(
                        out=prod[:], in0=h2[:, 0:N].bitcast(F32), scalar=1.0,
                        in1=ebc[:, 0:N], op0=ALU.mult, op1=ALU.mult,
                        accum_out=numall[:, s : s + 1],
                    )
                else:
                    nc.vector.tensor_mul(
                        prod[:], h2[:, 0:N].bitcast(F32), ebc[:, 0:N]
                    )
                    nc.vector.tensor_reduce(
                        out=numall[:, s : s + 1], in_=prod[:],
                        axis=mybir.AxisListType.X, op=ALU.add,
                    )

            if stage == 1:
                nc.sync.dma_start(out=out[0:1, 0:48], in_=dacc[0:1, 0:48])
                return

            # dall[s] = sum of the 3 per-chunk exp accumulators
            dall = small.tile([128, BL], F32, tag="dall")
            nc.vector.tensor_reduce(
                out=dall[:].unsqueeze(2),
                in_=dacc[:].rearrange("p (s c) -> p s c", c=3),
                axis=mybir.AxisListType.X,
                op=ALU.add,
            )

            if stage == 2:
                nc.sync.dma_start(out=out[0:1, 0:BL], in_=dall[0:1, :])
                return

            # ---- out_enc encoder: henc = relu(oxx @ Wenc.T + benc) [128, BL]
            # (emitted AFTER the main loop: engine queues dispatch in program
            # order, and putting these first stalled fc1 behind the late
            # consts DMAs)
            enc_ps = psum_small.tile([128, BL], F32, tag="sps")
            wenc = consts_sb[0:8, C_WENC : C_WENC + 64]
            nc.tensor.matmul(
                enc_ps[0:64, :], wenc, oxx_sb[:, 0:BL], start=True, stop=True
            )
            nc.tensor.matmul(
                enc_ps[64:128, :], wenc, oxx_sb[:, BL:B], start=True, stop=True
            )
            henc = small.tile([128, BL], F32, tag="henc")
            nc.scalar.activation(
                out=henc[:],
                in_=enc_ps[:],
                func=AF.Relu,
                bias=consts_sb[:, C_BENC : C_BENC + 1],
                scale=1.0,
            )

            # ---- repack the two 64-partition blocks into [64, B] tiles on
            # partitions 0:63.  A matmul whose MOVING operand starts at
            # partition 64 faults this device (measured, exp_repro L12/L14),
            # so the whole tail runs on partitions 0:63; block B halves are
            # moved down with small SBUF->SBUF DMAs (cross-partition moves
            # are DMA-only anyway).
            henc2 = small.tile([64, B], F32, tag="henc2")
            numall2 = small.tile([64, B], F32, tag="numall2")
            dall2 = small.tile([64, B], F32, tag="dall2")
            nc.scalar.copy(out=henc2[:, 0:BL], in_=henc[0:64, :])
            nc.sync.dma_start(out=henc2[:, BL:B], in_=henc[64:128, :])
            nc.scalar.copy(out=numall2[:, 0:BL], in_=numall[0:64, :])
            nc.sync.dma_start(out=numall2[:, BL:B], in_=numall[64:128, :])
            nc.scalar.copy(out=dall2[:, 0:BL], in_=dall[0:64, :])
            nc.sync.dma_start(out=dall2[:, BL:B], in_=dall[64:128, :])

            # e_enc = exp(wk . henc) replicated over the 64 partitions
            ee_ps = psum_small.tile([64, B], F32, tag="sps")
            wke64 = consts_sb[0:64, C_WKEF : C_WKEF + 64]
            nc.tensor.matmul(ee_ps[:], wke64, henc2[:], start=True, stop=True)
            eenc = small.tile([64, B], F32, tag="eenc")
            nc.scalar.activation(
                out=eenc[:], in_=ee_ps[:], func=AF.Exp, bias=0.0, scale=1.0
            )
            prod_enc = small.tile([64, B], F32, tag="prod_enc")
            nc.vector.tensor_mul(prod_enc[:], henc2[:], eenc[:])

            # add the out_enc node's contribution, then hbar = num / d
            dtot = small.tile([64, B], F32, tag="dtot")
            numtot = small.tile([64, B], F32, tag="numtot")
            nc.vector.tensor_add(dtot[:], dall2[:], eenc[:])
            nc.vector.tensor_add(numtot[:], numall2[:], prod_enc[:])
            rd = small.tile([64, B], F32, tag="rd")
            nc.vector.reciprocal(out=rd[:], in_=dtot[:])
            hbar = small.tile([64, B], F32, tag="hbar")
            nc.vector.tensor_mul(hbar[:], numtot[:], rd[:])

            if stage == 3:
                nc.sync.dma_start(out=out[0:1, 0:B], in_=hbar[0:1, :])
                return

            # ---- z = hbar @ (WA@WV).T + bva   -> [64, B] ----
            va0 = consts_sb[0:64, C_WVA : C_WVA + 64]
            z_ps = psum_small.tile([64, B], F32, tag="sps")
            nc.tensor.matmul(z_ps[:], va0, hbar[:], start=True, stop=True)
            bva = consts_sb[0:64, C_BVA : C_BVA + 1]
            zT = small.tile([64, B], F32, tag="zT")
            nc.scalar.activation(
                out=zT[:], in_=z_ps[:], func=AF.Identity, bias=bva, scale=1.0
            )

            if stage == 4:
                nc.sync.dma_start(out=out[0:1, 0:B], in_=zT[0:1, :])
                return

            # ---- tail: 4x (BN + relu), 3x linear, heads ----
            eps_col = small.tile([64, 1], F32, tag="eps")
            nc.vector.memset(eps_col[:], BN_EPS)
            gamma = consts_sb[0:64, C_GAMMA : C_GAMMA + 1]
            beta = consts_sb[0:64, C_BETA : C_BETA + 1]

            cur = zT
            node = None
            for r in range(4):
                st6 = small.tile([64, 6], F32, tag=f"st6_{r}")
                mv = small.tile([64, 2], F32, tag=f"mv_{r}")
                nc.vector.bn_stats(out=st6[:], in_=cur[:])
                nc.vector.bn_aggr(out=mv[:], in_=st6[:])
                sd = small.tile([64, 1], F32, tag=f"sd_{r}")
                nc.scalar.activation(
                    out=sd[:], in_=mv[:, 1:2], func=AF.Sqrt, bias=eps_col[:], scale=1.0
                )
                rstd = small.tile([64, 1], F32, tag=f"rstd_{r}")
                nc.vector.reciprocal(out=rstd[:], in_=sd[:])
                a = small.tile([64, 1], F32, tag=f"a_{r}")
                nc.vector.tensor_mul(a[:], rstd[:], gamma)
                mc = small.tile([64, 1], F32, tag=f"mc_{r}")
                nc.vector.tensor_mul(mc[:], mv[:, 0:1], a[:])
                cb = small.tile([64, 1], F32, tag=f"cb_{r}")
                nc.vector.tensor_sub(cb[:], beta, mc[:])
                node = small.tile([64, B], F32, tag=f"node_{r}")
                nc.scalar.activation(
                    out=node[:], in_=cur[:], func=AF.Relu, bias=cb[:], scale=a[:]
                )
                if r < 3:
                    zp = psum_small.tile([64, B], F32, tag="sps")
                    nc.tensor.matmul(zp[:], va0, node[:], start=True, stop=True)
                    nxt = small.tile([64, B], F32, tag=f"z_{r + 1}")
                    nc.scalar.activation(
                        out=nxt[:], in_=zp[:], func=AF.Identity, bias=bva, scale=1.0
                    )
                    cur = nxt

            # ---- heads (everything on partition 0: mu cols 0-31, sig 32-63) ----
            hp_mu = psum_small.tile([1, B], F32, tag="sps")
            nc.tensor.matmul(
                hp_mu[:],
                consts_sb[0:64, C_HEADS : C_HEADS + 1],
                node[:],
                start=True,
                stop=True,
            )
            hp_sig = psum_small.tile([1, B], F32, tag="sps")
            nc.tensor.matmul(
                hp_sig[:],
                consts_sb[0:64, C_HEADS + 1 : C_HEADS + 2],
                node[:],
                start=True,
                stop=True,
            )
            out_sb = small.tile([1, 2 * B], F32, tag="out_sb")
            nc.scalar.activation(
                out=out_sb[0:1, 0:B],
                in_=hp_mu[:],
                func=AF.Identity,
                bias=consts_sb[0:1, C_BMU : C_BMU + 1],
                scale=1.0,
            )
            sig_t = small.tile([1, B], F32, tag="sig_t")
            nc.scalar.activation(
                out=sig_t[:],
                in_=hp_sig[:],
                func=AF.Square,
                bias=consts_sb[0:1, C_BSIG : C_BSIG + 1],
                scale=1.0,
            )
            nc.vector.tensor_scalar_add(out_sb[0:1, B : 2 * B], sig_t[:], 0.01)
            nc.sync.dma_start(out=out, in_=out_sb[:])

          _body()

    nc.compile()
    return nc


def make_consts(inp):
    f32 = np.float32
    W1 = np.asarray(inp["W1"], f32)
    b1 = np.asarray(inp["b1"], f32)
    W2 = np.asarray(inp["W2"], f32)
    b2 = np.asarray(inp["b2"], f32)
    Wenc = np.asarray(inp["Wenc"], f32)
    benc = np.asarray(inp["benc"], f32)
    WK = np.asarray(inp["WK"], f32)
    WV = np.asarray(inp["WV"], f32)
    bV = np.asarray(inp["bV"], f32)
    wQKk = np.asarray(inp["wQKk"], f32)
    WA = np.asarray(inp["WA"], f32)
    bA = np.asarray(inp["bA"], f32)
    gamma = np.asarray(inp["gamma"], f32)
    beta = np.asarray(inp["beta"], f32)
    Wmu = np.asarray(inp["Wmu"], f32)
    bmu = np.asarray(inp["bmu"], f32)
    Wsig = np.asarray(inp["Wsig"], f32)
    bsig = np.asarray(inp["bsig"], f32)

    wk = WK.T @ wQKk  # [H]
    Wva = WA @ WV  # [H,H]
    bva = WA @ bV + bA

    cr = np.zeros((128, CWR), f32)
    cr[0:9, CR_W1 : CR_W1 + 64] = W1.T
    cr[9:18, CR_W1 + 64 : CR_W1 + 128] = W1.T
    cr[0:64, CR_W2 : CR_W2 + 64] = W2.T
    cr[64:128, CR_W2 + 64 : CR_W2 + 128] = W2.T
    cr[0:64, CR_WKE : CR_WKE + 64] = wk[:, None]
    cr[64:128, CR_WKE + 64 : CR_WKE + 128] = wk[:, None]

    c = np.zeros((128, CW), f32)
    c[0:64, C_WVA : C_WVA + 64] = Wva.T
    c[64:128, C_WVA : C_WVA + 64] = Wva.T
    c[0:8, C_WENC : C_WENC + 64] = Wenc.T
    c[0:64, C_WKEF : C_WKEF + 64] = wk[:, None]
    c[64:128, C_WKEF + 64 : C_WKEF + 128] = wk[:, None]
    c[0:64, C_HEADS] = Wmu[0]
    c[0:64, C_HEADS + 1] = Wsig[0]
    c[0:64, C_B1] = b1
    c[64:128, C_B1] = b1
    c[0:64, C_B2] = b2
    c[64:128, C_B2] = b2
    c[0:64, C_GAMMA] = gamma
    c[0:64, C_BETA] = beta
    c[0:64, C_BVA] = bva
    c[0:64, C_BENC] = benc
    c[64:128, C_BENC] = benc
    c[0, C_BMU] = bmu[0]
    c[0, C_BSIG] = bsig[0]
    return cr, c


def make_in_maps(inputs):
    f32 = np.float32
    xx = np.asarray(inputs["input_xx"], f32)  # [B, N, 8]
    yy = np.asarray(inputs["input_yy"], f32)  # [B, N]
    oxx = np.asarray(inputs["output_xx"], f32)  # [B, 1, 8]

    xcat = np.zeros((18, BL, NN), f32)
    xcat[0:8, :, 0:N] = xx[0:BL].transpose(2, 0, 1)
    xcat[8, :, 0:N] = yy[0:BL]
    xcat[9:17, :, 0:N] = xx[BL:B].transpose(2, 0, 1)
    xcat[17, :, 0:N] = yy[BL:B]
    xcat = np.ascontiguousarray(xcat.reshape(18, R))

    oxxT = np.ascontiguousarray(oxx[:, 0, :].T)  # [8, B]
    constsr, consts = make_consts(inputs)
    return [{"xcat": xcat, "oxxT": oxxT, "constsr": constsr, "consts": consts}]


_NC_CACHE = {}


def get_nc():
    if "nc" not in _NC_CACHE:
        import os

        stage = int(os.environ.get("KERNEL_STAGE", "99"))
        use_ttr = os.environ.get("KERNEL_TTR", "1") == "1"
        use_acc = os.environ.get("KERNEL_ACC", "1") == "1"
        use_stt = os.environ.get("KERNEL_STT", "1") == "1"
        nsc = int(os.environ.get("KERNEL_NSC", str(BL)))
        _NC_CACHE["nc"] = build_nc(
            stage=stage, use_ttr=use_ttr, use_acc=use_acc, use_stt=use_stt,
            nsc=nsc,
        )
    return _NC_CACHE["nc"]


def kernel(**inputs):
    nc = get_nc()
    in_maps = make_in_maps(inputs)
    res = bass_utils.run_bass_kernel_spmd(nc, in_maps, core_ids=[0])
    out = np.asarray(res.results[0]["out"], np.float32).reshape(2 * B)
    mu_out = out[0:B].reshape(B, 1).copy()
    sig_out = out[B : 2 * B].reshape(B, 1).copy()
    return mu_out, sig_out


# revision 17
# speedup vs baseline: 1.8855x; 1.3511x over previous
"""Trainium2 Bass kernel for the GNN message-passing problem.

Math notes (why this is exact, not an approximation):
  score[i,b,j] = q[i,b]@wQKq + k[j,b]@wQKk + bQK.  Softmax over j is
  invariant to terms constant in j, so the attention weights are
  p[b,j] = softmax_j(nodes[j,b]@(WK.T@wQKk)) -- independent of the query
  node i.  Hence every node receives the same aggregated message and all
  Nn nodes are identical after one communicate() round; the final
  max-over-nodes is the common value.  Since p sums to 1 and V/A2N are
  affine, aggre@WV.T@WA.T collapses to hbar@(WA@WV).T with
  hbar[b] = sum_j p[b,j]*nodes[j,b].  Rounds 2-4 operate on identical
  nodes (uniform softmax == identity mix) and reduce to [B,H] math.

Execution strategy: single NeuronCore.  Multi-core was measured and
rejected: an 8-core (or even 2-core) AllReduce/AllGather through the
collectives runtime has a ~70-90us mechanism floor on this setup --
more than this whole kernel.

Perf structure (vs the 160us fp32 predecessor):
  * All three big matmul passes (fc1, fc2, score) run in float32r --
    1 PE cycle/column instead of 4 for fp32 (ap>=256), at ~1.6e-4
    matmul error (measured) vs ~4e-3 for bf16.  h1/h2 live in f32r
    tiles; their producers round on write, which walrus requires.
  * Softmax denominator rides the Exp drain for free via activation
    accum_out; the numerator sum_j ebc*h2 is one fused DVE
    tensor_tensor_reduce pass instead of tensor_mul + tensor_reduce.
  * PSUM drains are spread across the three non-PE compute engines so
    no engine exceeds ~2.6us per 1536-col superchunk: Scalar does the
    3 Exp chunks, DVE does one fc2-relu chunk + the fused reduce,
    Pool/gpsimd does the other 5 relu chunks (scalar_tensor_tensor:
    relu(psum + bias) = (ps add b) max zeros).

On-chip layout: transposed activations [feature, col] with two
64-feature blocks on the 128 partitions.  Block A (partitions 0-63)
holds batches 0-15, block B (partitions 64-127) batches 16-31;
1536 columns per batch (1535 input nodes + 1 unused scratch column --
the out_enc node is handled out-of-band so the big reductions run on
a clean 1535-column span).

The [18, 24576] fc1 input (x||y, both blocks) is assembled on the HOST
so the device DMA is fully contiguous.
"""

import numpy as np

try:
    import concourse.bass as bass  # noqa: F401
except ImportError:  # pragma: no cover - container default path
    import sys

    sys.path.insert(0, "/opt/trn_rl_repo")
    import concourse.bass as bass  # noqa: F401

import concourse.bacc as bacc
import concourse.tile as tile
from concourse import mybir
from concourse import bass_utils

F32 = mybir.dt.float32
F32R = mybir.dt.float32r
AF = mybir.ActivationFunctionType
ALU = mybir.AluOpType

B = 32
BL = B // 2  # 16 batches per 64-partition block
N = 1535  # input nodes
NN = N + 1  # +1 col per batch (scratch; out_enc handled separately)
H = 64
R = BL * NN  # 24576 free columns
CH = 512  # PSUM-bank-sized matmul chunk
BN_EPS = 1e-5

# f32r consts tensor column map ([128, CWR])
CR_W1 = 0  # rows 0-8 cols 0-63 = W1.T ; rows 9-17 cols 64-127 = W1.T
CR_W2 = 128  # [128,128] blockdiag(W2.T, W2.T)
CR_WKE = 256  # [128,128] blockdiag(wk x ones, wk x ones)
CWR = 384

# fp32 consts tensor column map ([128, CW])
C_WVA = 0  # [128,64] rows 0-63 = (WA@WV).T ; rows 64-127 = same
C_WENC = 64  # [128,64] rows 0-7 = Wenc.T
C_WKEF = 128  # [128,128] fp32 blockdiag(wk x ones) for the enc path
C_HEADS = 256  # [64,2] col0 = Wmu[0], col1 = Wsig[0]
C_B1 = 258  # [128,1] concat(b1,b1)
C_B2 = 259  # [128,1] concat(b2,b2)
C_GAMMA = 260  # [64,1]
C_BETA = 261  # [64,1]
C_BVA = 262  # [64,1] WA@bV + bA
C_BENC = 263  # [128,1] concat(benc,benc)
C_BMU = 264  # row0 = bmu
C_BSIG = 265  # row0 = bsig
CW = 266


def build_nc(stage=99, use_ttr=True, use_acc=True, use_stt=True, nsc=BL):
    """stage caps how much of the pipeline is emitted (debug bisect aid);
    the output tensor is written with whatever is available at that stage.
    use_ttr/use_acc disable the fused DVE reduce / activation accum_out."""
    nc = bacc.Bacc(
        "TRN2",
        target_bir_lowering=False,
        debug=False,
        enable_asserts=True,
        num_devices=1,
    )
    xcat = nc.dram_tensor("xcat", [18, R], F32R, kind="ExternalInput").ap()
    oxxT = nc.dram_tensor("oxxT", [8, B], F32, kind="ExternalInput").ap()
    constsr = nc.dram_tensor("constsr", [128, CWR], F32R, kind="ExternalInput").ap()
    consts = nc.dram_tensor("consts", [128, CW], F32, kind="ExternalInput").ap()
    out = nc.dram_tensor("out", [1, 2 * B], F32, kind="ExternalOutput").ap()

    with tile.TileContext(nc) as tc:
        with (
            tc.tile_pool(name="big", bufs=1) as big,
            tc.tile_pool(name="small", bufs=1) as small,
            tc.tile_pool(name="h1p", bufs=3) as h1p,
            tc.tile_pool(name="h2p", bufs=3) as h2p,
            tc.tile_pool(name="ep", bufs=3) as ep,
            tc.tile_pool(name="prodp", bufs=2) as prodp,
            tc.tile_pool(name="psum_big", bufs=6, space="PSUM") as psum_big,
            tc.tile_pool(name="psum_small", bufs=2, space="PSUM") as psum_small,
        ):
          def _body():
            # ---- input DMAs (ordered by when the main loop needs them) ----
            constsr_sb = big.tile([128, CWR], F32R, tag="constsr")
            consts_sb = big.tile([128, CW], F32, tag="consts")
            xT = big.tile([18, R], F32R, tag="xT")
            nc.sync.dma_start(
                out=constsr_sb[:, CR_W1 : CR_W1 + 128],
                in_=constsr[:, CR_W1 : CR_W1 + 128],
            )
            nc.sync.dma_start(
                out=consts_sb[:, C_HEADS:CW], in_=consts[:, C_HEADS:CW]
            )
            nc.sync.dma_start(out=xT[:, 0:NN], in_=xcat[:, 0:NN])
            nc.sync.dma_start(
                out=constsr_sb[:, CR_W2 : CR_W2 + 128],
                in_=constsr[:, CR_W2 : CR_W2 + 128],
            )
            nc.sync.dma_start(
                out=constsr_sb[:, CR_WKE : CR_WKE + 128],
                in_=constsr[:, CR_WKE : CR_WKE + 128],
            )
            for s in range(1, BL):
                sp = slice(s * NN, (s + 1) * NN)
                nc.sync.dma_start(out=xT[:, sp], in_=xcat[:, sp])
            nc.sync.dma_start(
                out=consts_sb[:, C_WVA:C_HEADS], in_=consts[:, C_WVA:C_HEADS]
            )
            oxx_sb = small.tile([8, B], F32, tag="oxx")
            nc.sync.dma_start(out=oxx_sb[:], in_=oxxT)

            # ---- main pipeline: one superchunk per batch-pair (1536 cols) ----
            zeros = small.tile([128, CH], F32, tag="zeros")
            nc.vector.memset(zeros[:], 0.0)
            dacc = small.tile([128, 3 * BL], F32, tag="dacc")
            numall = small.tile([128, BL], F32, tag="numall")
            w1blk = constsr_sb[0:18, CR_W1 : CR_W1 + 128]
            w2blk = constsr_sb[:, CR_W2 : CR_W2 + 128]
            wkeblk = constsr_sb[:, CR_WKE : CR_WKE + 128]
            b1 = consts_sb[:, C_B1 : C_B1 + 1]
            b2 = consts_sb[:, C_B2 : C_B2 + 1]
            # Software-pipelined main loop: phase ph runs fc1 for superchunk
            # ph, fc2 for ph-1, and score/exp/num for ph-2.  The PE executes
            # matmuls in program order, so interleaving the three stages of
            # DIFFERENT superchunks keeps every matmul's moving operand
            # drained ~a full phase earlier -- the PE never sits behind a
            # just-issued drain (that serialization cost the naive loop
            # ~9us per superchunk; back-to-back matmuls sustain 272ns each).
            #
            # [128,512] single-bank PSUM tiles throughout: walrus rejects
            # multi-bank matmul outputs, and engine reads that span PSUM
            # banks fault the device.
            h1_t = [None] * nsc
            h2_t = [None] * nsc
            for ph in range(nsc + 2):
                if ph < nsc:
                    s = ph
                    base = s * NN
                    h1 = h1p.tile([128, NN], F32R, tag="h1")
                    h1_t[s] = h1
                    for c in range(3):
                        sp = slice(c * CH, (c + 1) * CH)
                        ps = psum_big.tile([128, CH], F32, tag="mm")
                        nc.tensor.matmul(
                            ps[:],
                            w1blk,
                            xT[:, base + c * CH : base + (c + 1) * CH],
                            start=True,
                            stop=True,
                        )
                        # drains split between DVE and Scalar (the only
                        # engines with PSUM access), balanced 3/3 per phase
                        if c < 2 and use_stt:
                            nc.vector.scalar_tensor_tensor(
                                out=h1[:, sp], in0=ps[:], scalar=b1,
                                in1=zeros[:], op0=ALU.add, op1=ALU.max,
                            )
                        else:
                            nc.scalar.activation(
                                out=h1[:, sp], in_=ps[:], func=AF.Relu,
                                bias=b1, scale=1.0,
                            )
                if 1 <= ph <= nsc:
                    s = ph - 1
                    h1 = h1_t[s]
                    h2 = h2p.tile([128, NN], F32R, tag="h2")
                    h2_t[s] = h2
                    for c in range(3):
                        sp = slice(c * CH, (c + 1) * CH)
                        ps = psum_big.tile([128, CH], F32, tag="mm")
                        nc.tensor.matmul(
                            ps[:], w2blk, h1[:, sp], start=True, stop=True
                        )
                        if c < 1 and use_stt:
                            nc.vector.scalar_tensor_tensor(
                                out=h2[:, sp], in0=ps[:], scalar=b2,
                                in1=zeros[:], op0=ALU.add, op1=ALU.max,
                            )
                        else:
                            nc.scalar.activation(
                                out=h2[:, sp], in_=ps[:], func=AF.Relu,
                                bias=b2, scale=1.0,
                            )
                if ph >= 2:
                    s = ph - 2
                    h2 = h2_t[s]
                    ebc = ep.tile([128, NN], F32, tag="ebc")
                    for c in range(3):
                        ncol = CH if c < 2 else CH - 1  # skip scratch col
                        sp = slice(c * CH, c * CH + ncol)
                        ps = psum_big.tile([128, CH], F32, tag="mm")
                        nc.tensor.matmul(
                            ps[:], wkeblk, h2[:, c * CH : (c + 1) * CH],
                            start=True, stop=True,
                        )
                        if use_acc:
                            nc.scalar.activation(
                                out=ebc[:, sp], in_=ps[:, 0:ncol], func=AF.Exp,
                                bias=0.0, scale=1.0,
                                accum_out=dacc[:, 3 * s + c : 3 * s + c + 1],
                            )
                        else:
                            nc.scalar.activation(
                                out=ebc[:, sp], in_=ps[:, 0:ncol], func=AF.Exp,
                                bias=0.0, scale=1.0,
                            )
                    if not use_acc:
                        nc.vector.tensor_reduce(
                            out=dacc[:, 3 * s : 3 * s + 1], in_=ebc[:, 0:N],
                            axis=mybir.AxisListType.X, op=ALU.add,
                        )
                        nc.vector.memset(dacc[:, 3 * s + 1 : 3 * s + 3], 0.0)
                    # num[b] = sum_j ebc[j]*h2[j] in one fused DVE pass.
                    # NOTE: tensor_tensor_reduce faults the HW (measured);
                    # scalar_tensor_tensor with accum_out is the working
                    # fused form: out = (h2 * 1.0) * ebc, accum = sum(out).
                    prod = prodp.tile([128, N], F32, tag="prod")
                    if use_ttr:
                        nc.vector.scalar_tensor_tensor(
                            out=prod[:], in0=h2[:, 0:N].bitcast(F32),
                            scalar=1.0, in1=ebc[:, 0:N],
                            op0=ALU.mult, op1=ALU.mult,
                            accum_out=numall[:, s : s + 1],
                        )
                    else:
                        nc.vector.tensor_mul(
                            prod[:], h2[:, 0:N].bitcast(F32), ebc[:, 0:N]
                        )
                        nc.vector.tensor_reduce(
                            out=numall[:, s : s + 1], in_=prod[:],
                            axis=mybir.AxisListType.X, op=ALU.add,
                        )

            if stage == 1:
                nc.sync.dma_start(out=out[0:1, 0:48], in_=dacc[0:1, 0:48])
                return

            # dall[s] = sum of the 3 per-chunk exp accumulators
            dall = small.tile([128, BL], F32, tag="dall")
            nc.vector.tensor_reduce(
                out=dall[:].unsqueeze(2),
                in_=dacc[:].rearrange("p (s c) -> p s c", c=3),
                axis=mybir.AxisListType.X,
                op=ALU.add,
            )

            if stage == 2:
                nc.sync.dma_start(out=out[0:1, 0:BL], in_=dall[0:1, :])
                return

            # ---- out_enc encoder: henc = relu(oxx @ Wenc.T + benc) [128, BL]
            # (emitted AFTER the main loop: engine queues dispatch in program
            # order, and putting these first stalled fc1 behind the late
            # consts DMAs)
            enc_ps = psum_small.tile([128, BL], F32, tag="sps")
            wenc = consts_sb[0:8, C_WENC : C_WENC + 64]
            nc.tensor.matmul(
                enc_ps[0:64, :], wenc, oxx_sb[:, 0:BL], start=True, stop=True
            )
            nc.tensor.matmul(
                enc_ps[64:128, :], wenc, oxx_sb[:, BL:B], start=True, stop=True
            )
            henc = small.tile([128, BL], F32, tag="henc")
            nc.scalar.activation(
                out=henc[:],
                in_=enc_ps[:],
                func=AF.Relu,
                bias=consts_sb[:, C_BENC : C_BENC + 1],
                scale=1.0,
            )

            # ---- repack the two 64-partition blocks into [64, B] tiles on
            # partitions 0:63.  A matmul whose MOVING operand starts at
            # partition 64 faults this device (measured, exp_repro L12/L14),
            # so the whole tail runs on partitions 0:63; block B halves are
            # moved down with small SBUF->SBUF DMAs (cross-partition moves
            # are DMA-only anyway).
            henc2 = small.tile([64, B], F32, tag="henc2")
            numall2 = small.tile([64, B], F32, tag="numall2")
            dall2 = small.tile([64, B], F32, tag="dall2")
            nc.scalar.copy(out=henc2[:, 0:BL], in_=henc[0:64, :])
            nc.sync.dma_start(out=henc2[:, BL:B], in_=henc[64:128, :])
            nc.scalar.copy(out=numall2[:, 0:BL], in_=numall[0:64, :])
            nc.sync.dma_start(out=numall2[:, BL:B], in_=numall[64:128, :])
            nc.scalar.copy(out=dall2[:, 0:BL], in_=dall[0:64, :])
            nc.sync.dma_start(out=dall2[:, BL:B], in_=dall[64:128, :])

            # e_enc = exp(wk . henc) replicated over the 64 partitions
            ee_ps = psum_small.tile([64, B], F32, tag="sps")
            wke64 = consts_sb[0:64, C_WKEF : C_WKEF + 64]
            nc.tensor.matmul(ee_ps[:], wke64, henc2[:], start=True, stop=True)
            eenc = small.tile([64, B], F32, tag="eenc")
            nc.scalar.activation(
                out=eenc[:], in_=ee_ps[:], func=AF.Exp, bias=0.0, scale=1.0
            )
            prod_enc = small.tile([64, B], F32, tag="prod_enc")
            nc.vector.tensor_mul(prod_enc[:], henc2[:], eenc[:])

            # add the out_enc node's contribution, then hbar = num / d
            dtot = small.tile([64, B], F32, tag="dtot")
            numtot = small.tile([64, B], F32, tag="numtot")
            nc.vector.tensor_add(dtot[:], dall2[:], eenc[:])
            nc.vector.tensor_add(numtot[:], numall2[:], prod_enc[:])
            rd = small.tile([64, B], F32, tag="rd")
            nc.vector.reciprocal(out=rd[:], in_=dtot[:])
            hbar = small.tile([64, B], F32, tag="hbar")
            nc.vector.tensor_mul(hbar[:], numtot[:], rd[:])

            if stage == 3:
                nc.sync.dma_start(out=out[0:1, 0:B], in_=hbar[0:1, :])
                return

            # ---- z = hbar @ (WA@WV).T + bva   -> [64, B] ----
            va0 = consts_sb[0:64, C_WVA : C_WVA + 64]
            z_ps = psum_small.tile([64, B], F32, tag="sps")
            nc.tensor.matmul(z_ps[:], va0, hbar[:], start=True, stop=True)
            bva = consts_sb[0:64, C_BVA : C_BVA + 1]
            zT = small.tile([64, B], F32, tag="zT")
            nc.scalar.activation(
                out=zT[:], in_=z_ps[:], func=AF.Identity, bias=bva, scale=1.0
            )

            if stage == 4:
                nc.sync.dma_start(out=out[0:1, 0:B], in_=zT[0:1, :])
                return

            # ---- tail: 4x (BN + relu), 3x linear, heads ----
            eps_col = small.tile([64, 1], F32, tag="eps")
            nc.vector.memset(eps_col[:], BN_EPS)
            gamma = consts_sb[0:64, C_GAMMA : C_GAMMA + 1]
            beta = consts_sb[0:64, C_BETA : C_BETA + 1]

            cur = zT
            node = None
            for r in range(4):
                st6 = small.tile([64, 6], F32, tag=f"st6_{r}")
                mv = small.tile([64, 2], F32, tag=f"mv_{r}")
                nc.vector.bn_stats(out=st6[:], in_=cur[:])
                nc.vector.bn_aggr(out=mv[:], in_=st6[:])
                sd = small.tile([64, 1], F32, tag=f"sd_{r}")
                nc.scalar.activation(
                    out=sd[:], in_=mv[:, 1:2], func=AF.Sqrt, bias=eps_col[:], scale=1.0
                )
                rstd = small.tile([64, 1], F32, tag=f"rstd_{r}")
                nc.vector.reciprocal(out=rstd[:], in_=sd[:])
                a = small.tile([64, 1], F32, tag=f"a_{r}")
                nc.vector.tensor_mul(a[:], rstd[:], gamma)
                mc = small.tile([64, 1], F32, tag=f"mc_{r}")
                nc.vector.tensor_mul(mc[:], mv[:, 0:1], a[:])
                cb = small.tile([64, 1], F32, tag=f"cb_{r}")
                nc.vector.tensor_sub(cb[:], beta, mc[:])
                node = small.tile([64, B], F32, tag=f"node_{r}")
                nc.scalar.activation(
                    out=node[:], in_=cur[:], func=AF.Relu, bias=cb[:], scale=a[:]
                )
                if r < 3:
                    zp = psum_small.tile([64, B], F32, tag="sps")
                    nc.tensor.matmul(zp[:], va0, node[:], start=True, stop=True)
                    nxt = small.tile([64, B], F32, tag=f"z_{r + 1}")
                    nc.scalar.activation(
                        out=nxt[:], in_=zp[:], func=AF.Identity, bias=bva, scale=1.0
                    )
                    cur = nxt

            # ---- heads (everything on partition 0: mu cols 0-31, sig 32-63) ----
            hp_mu = psum_small.tile([1, B], F32, tag="sps")
            nc.tensor.matmul(
                hp_mu[:],
                consts_sb[0:64, C_HEADS : C_HEADS + 1],
                node[:],
                start=True,
                stop=True,
            )
            hp_sig = psum_small.tile([1, B], F32, tag="sps")
            nc.tensor.matmul(
                hp_sig[:],
                consts_sb[0:64, C_HEADS + 1 : C_HEADS + 2],
                node[:],
                start=True,
                stop=True,
            )
            out_sb = small.tile([1, 2 * B], F32, tag="out_sb")
            nc.scalar.activation(
                out=out_sb[0:1, 0:B],
                in_=hp_mu[:],
                func=AF.Identity,
                bias=consts_sb[0:1, C_BMU : C_BMU + 1],
                scale=1.0,
            )
            sig_t = small.tile([1, B], F32, tag="sig_t")
            nc.scalar.activation(
                out=sig_t[:],
                in_=hp_sig[:],
                func=AF.Square,
                bias=consts_sb[0:1, C_BSIG : C_BSIG + 1],
                scale=1.0,
            )
            nc.vector.tensor_scalar_add(out_sb[0:1, B : 2 * B], sig_t[:], 0.01)
            nc.sync.dma_start(out=out, in_=out_sb[:])

          _body()

    nc.compile()
    return nc


def make_consts(inp):
    f32 = np.float32
    W1 = np.asarray(inp["W1"], f32)
    b1 = np.asarray(inp["b1"], f32)
    W2 = np.asarray(inp["W2"], f32)
    b2 = np.asarray(inp["b2"], f32)
    Wenc = np.asarray(inp["Wenc"], f32)
    benc = np.asarray(inp["benc"], f32)
    WK = np.asarray(inp["WK"], f32)
    WV = np.asarray(inp["WV"], f32)
    bV = np.asarray(inp["bV"], f32)
    wQKk = np.asarray(inp["wQKk"], f32)
    WA = np.asarray(inp["WA"], f32)
    bA = np.asarray(inp["bA"], f32)
    gamma = np.asarray(inp["gamma"], f32)
    beta = np.asarray(inp["beta"], f32)
    Wmu = np.asarray(inp["Wmu"], f32)
    bmu = np.asarray(inp["bmu"], f32)
    Wsig = np.asarray(inp["Wsig"], f32)
    bsig = np.asarray(inp["bsig"], f32)

    wk = WK.T @ wQKk  # [H]
    Wva = WA @ WV  # [H,H]
    bva = WA @ bV + bA

    cr = np.zeros((128, CWR), f32)
    cr[0:9, CR_W1 : CR_W1 + 64] = W1.T
    cr[9:18, CR_W1 + 64 : CR_W1 + 128] = W1.T
    cr[0:64, CR_W2 : CR_W2 + 64] = W2.T
    cr[64:128, CR_W2 + 64 : CR_W2 + 128] = W2.T
    cr[0:64, CR_WKE : CR_WKE + 64] = wk[:, None]
    cr[64:128, CR_WKE + 64 : CR_WKE + 128] = wk[:, None]

    c = np.zeros((128, CW), f32)
    c[0:64, C_WVA : C_WVA + 64] = Wva.T
    c[64:128, C_WVA : C_WVA + 64] = Wva.T
    c[0:8, C_WENC : C_WENC + 64] = Wenc.T
    c[0:64, C_WKEF : C_WKEF + 64] = wk[:, None]
    c[64:128, C_WKEF + 64 : C_WKEF + 128] = wk[:, None]
    c[0:64, C_HEADS] = Wmu[0]
    c[0:64, C_HEADS + 1] = Wsig[0]
    c[0:64, C_B1] = b1
    c[64:128, C_B1] = b1
    c[0:64, C_B2] = b2
    c[64:128, C_B2] = b2
    c[0:64, C_GAMMA] = gamma
    c[0:64, C_BETA] = beta
    c[0:64, C_BVA] = bva
    c[0:64, C_BENC] = benc
    c[64:128, C_BENC] = benc
    c[0, C_BMU] = bmu[0]
    c[0, C_BSIG] = bsig[0]
    return cr, c


def make_in_maps(inputs):
    f32 = np.float32
    xx = np.asarray(inputs["input_xx"], f32)  # [B, N, 8]
    yy = np.asarray(inputs["input_yy"], f32)  # [B, N]
    oxx = np.asarray(inputs["output_xx"], f32)  # [B, 1, 8]

    xcat = np.zeros((18, BL, NN), f32)
    xcat[0:8, :, 0:N] = xx[0:BL].transpose(2, 0, 1)
    xcat[8, :, 0:N] = yy[0:BL]
    xcat[9:17, :, 0:N] = xx[BL:B].transpose(2, 0, 1)
    xcat[17, :, 0:N] = yy[BL:B]
    xcat = np.ascontiguousarray(xcat.reshape(18, R))

    oxxT = np.ascontiguousarray(oxx[:, 0, :].T)  # [8, B]
    constsr, consts = make_consts(inputs)
    return [{"xcat": xcat, "oxxT": oxxT, "constsr": constsr, "consts": consts}]


_NC_CACHE = {}


def get_nc():
    if "nc" not in _NC_CACHE:
        import os

        stage = int(os.environ.get("KERNEL_STAGE", "99"))
        use_ttr = os.environ.get("KERNEL_TTR", "1") == "1"
        use_acc = os.environ.get("KERNEL_ACC", "1") == "1"
        use_stt = os.environ.get("KERNEL_STT", "1") == "1"
        nsc = int(os.environ.get("KERNEL_NSC", str(BL)))
        _NC_CACHE["nc"] = build_nc(
            stage=stage, use_ttr=use_ttr, use_acc=use_acc, use_stt=use_stt,
            nsc=nsc,
        )
    return _NC_CACHE["nc"]


def kernel(**inputs):
    nc = get_nc()
    in_maps = make_in_maps(inputs)
    res = bass_utils.run_bass_kernel_spmd(nc, in_maps, core_ids=[0])
    out = np.asarray(res.results[0]["out"], np.float32).reshape(2 * B)
    mu_out = out[0:B].reshape(B, 1).copy()
    sig_out = out[B : 2 * B].reshape(B, 1).copy()
    return mu_out, sig_out
